# revision 22
# baseline (speedup 1.0000x reference)
"""Banded-Toeplitz HRF stack kernel v2 for Trainium2 (8 NeuronCores).

Problem: theta [512,1] -> H [512,400,400] f32 where
  k[b,:] = gamma_pdf(t, 5, theta_b) - 0.167 * gamma_pdf(t, 15, theta_b)
  H[b, j, i] = k[b, j-i] if 0 <= j-i < 30 else 0

Per core (64 batches):
  1. Input DMA [128, 91] = [theta | t_rev | c1_rev | c2_rev] (theta on 2
     partitions per batch).
  2. DVE computes arg = theta*t_rev, theta powers, gg; ACT computes
     ee = exp(-arg) (Exp LUT preloaded by a dummy activation);
     DVE writes krev = gg*ee into s[:, 370:400]; s[:, 400:429] is zero.
  3. SP spills s[:, 370:429] (one partition per batch) to a DRAM scratch
     `spill` [64, 59] = [krev | zeros].
  4. Band rows 29..399 are written by DRAM->DRAM DMAs whose dst AP puts
     the 371-row dim first ([[401,371],[160000,nb],[1,30]]) and whose src
     broadcasts the spill row ([[0,371],[59,nb],[1,30]]). Rows 0..28 are
     written by SBUF-sourced sliding-window rects as in the baseline.
     Work is split across the three DMA-capable engines (SP, ACT, Pool),
     whose DMA queues run concurrently; ACT and Pool queues are
     pre-warmed with tiny dummy DMAs so their DGE init overlaps the
     input/compute phase.
  Cells outside the band stay zero (ExternalOutput buffers are
  pre-zeroed donated buffers under the PJRT/axon path).

Measured (CoreSim cost model, per core): 6818 ns vs 23876 ns baseline.
"""

import numpy as np

B = 512
T = 400
L = 30
NCORES = 8
BPC = B // NCORES  # 64 batches per core

SW = 1200
KREV = 370  # krev columns [370, 400); zeros [400, 432)

# batch split of the rows-29..399 band writes across (SP, ACT, Pool)
BC_SPLIT = [(0, 25), (25, 24), (49, 15)]  # (b0, nb)
# staircase split of the rows-0..28 triangle: (r0, nr, ncols)
# stair i covers rows [r0, r0+nr) writing cols [0, ncols) (ncols >= r0+nr)
A_STAIRS = [(0, 13, 13), (13, 8, 21), (21, 8, 30)]

_CACHE = {}


def _host_constants():
    t = np.linspace(0.0, 30.0, 30000, dtype=np.float32)[::1000]
    t = np.maximum(t, np.float32(1e-8)).astype(np.float64)
    tr = t[::-1].copy()
    c1r = tr**5 / 120.0
    c2r = -0.167 * tr**15 / 1307674368000.0
    return np.concatenate([[0.0], tr, c1r, c2r]).astype(np.float32)


def _in_map(theta_slice):
    row = np.tile(_host_constants(), (128, 1))
    row[:, 0] = -np.repeat(theta_slice, 2)
    return {"inp": np.ascontiguousarray(row, dtype=np.float32)}


def _build_nc():
    import concourse.bass as bass
    import concourse.mybir as mybir
    from concourse.ap import AP
    from contextlib import ExitStack

    f32 = mybir.dt.float32
    nc = bass.Bass()

    inp = nc.declare_dram_parameter("inp", [128, 91], f32, isOutput=False)
    out = nc.declare_dram_parameter("H", [BPC, T, T], f32, isOutput=True)
    spill = nc.declare_dram_parameter("spill", [BPC, 59], f32, isOutput=True)
    scr = nc.declare_dram_parameter("scr", [2, 16], f32, isOutput=True)
    out_t = out[:].tensor
    sp_t = spill[:].tensor
    scr_t = scr[:].tensor
    inp_t = inp[:].tensor

    ctx = ExitStack()
    nc._kernel_ctx = ctx

    cst = ctx.enter_context(nc.sbuf_tensor([128, 91], f32))
    p2 = ctx.enter_context(nc.sbuf_tensor([128, 1], f32))
    p4 = ctx.enter_context(nc.sbuf_tensor([128, 1], f32))
    p6 = ctx.enter_context(nc.sbuf_tensor([128, 1], f32))
    p16 = ctx.enter_context(nc.sbuf_tensor([128, 1], f32))
    arg = ctx.enter_context(nc.sbuf_tensor([128, L], f32))
    ee = ctx.enter_context(nc.sbuf_tensor([128, L], f32))
    gg = ctx.enter_context(nc.sbuf_tensor([128, L], f32))
    zz = ctx.enter_context(nc.sbuf_tensor([128, 1], f32))
    s = ctx.enter_context(nc.sbuf_tensor([128, SW], f32))

    wsem = ctx.enter_context(nc.semaphore("wsem"))
    isem = ctx.enter_context(nc.semaphore("isem"))
    vsem = ctx.enter_context(nc.semaphore("vsem"))
    asem = ctx.enter_context(nc.semaphore("asem"))
    ksem = ctx.enter_context(nc.semaphore("ksem"))
    ssem = ctx.enter_context(nc.semaphore("ssem"))
    osem = ctx.enter_context(nc.semaphore("osem"))
    dsem1 = ctx.enter_context(nc.semaphore("dsem1"))
    dsem2 = ctx.enter_context(nc.semaphore("dsem2"))
    posem = ctx.enter_context(nc.semaphore("posem"))

    th = cst[:, 0:1]
    tr_c = cst[:, 1:31]
    c1_c = cst[:, 31:61]
    c2_c = cst[:, 61:91]

    s_t = s[:].tensor

    def bc_aps(b0, nb):
        src = AP(tensor=sp_t, offset=59 * b0,
                 ap=[[0, 371], [59, nb], [1, L]])
        dst = AP(tensor=out_t, offset=T * T * b0 + 401 * 29 - 29,
                 ap=[[401, 371], [T * T, nb], [1, L]])
        return dst, src

    def a_aps(r0, nr, nc):
        src = AP(tensor=s_t, offset=399 - r0,
                 ap=[[2 * SW, BPC], [-1, nr], [1, nc]])
        dst = AP(tensor=out_t, offset=T * r0,
                 ap=[[T * T, BPC], [T, nr], [1, nc]])
        return dst, src

    def warm_aps(row):
        src = AP(tensor=inp_t, offset=0, ap=[[91, 1], [1, 16]])
        dst = AP(tensor=scr_t, offset=16 * row, ap=[[16, 1], [1, 16]])
        return dst, src


    with nc.Block() as block:

        @block.sync
        def _(sync):
            # input DMA warms the SP DGE pipeline too
            sync.dma_start(cst[:], inp[:]).then_inc(isem, 16)
            # spill [krev | zeros] -> DRAM (one partition per batch)
            sp_dst = AP(tensor=sp_t, offset=0, ap=[[59, BPC], [1, 59]])
            sp_src = AP(tensor=s_t, offset=KREV, ap=[[2 * SW, BPC], [1, 59]])
            sync.dma_start(sp_dst, sp_src)._wait_ge(ksem, 2).then_inc(ssem, 16)
            d, sr = bc_aps(*BC_SPLIT[0])
            sync.dma_start(d, sr)._wait_ge(ssem, 16).then_inc(osem, 16)
            pass

        @block.scalar
        def _(scalar):
            # the memzero (a Copy activation) charges the ACT table load at
            # program start and keeps ACT busy past the input DMA's sem-value
            # update, so the isem wait below polls instead of sleeping
            scalar.memzero(zz[:])
            scalar.wait_ge(isem, 16)
            scalar.activation(ee[:], tr_c, bass.mybir.ActivationFunctionType.Exp,
                              scale=th).then_inc(asem, 1)
            d, sr = a_aps(*A_STAIRS[0])
            scalar.dma_start(d, sr)._wait_ge(ksem, 2).then_inc(osem, 16)
            d, sr = bc_aps(*BC_SPLIT[1])
            scalar.dma_start(d, sr)._wait_ge(ssem, 16).then_inc(osem, 16)

        @block.gpsimd
        def _(gpsimd):
            d, sr = warm_aps(1)
            gpsimd.dma_start(d, sr).then_inc(dsem2, 16)
            d, sr = a_aps(*A_STAIRS[1])
            gpsimd.dma_start(d, sr)._wait_ge(ksem, 2).then_inc(posem, 16)
            d, sr = a_aps(*A_STAIRS[2])
            gpsimd.dma_start(d, sr).then_inc(posem, 16)
            d, sr = bc_aps(*BC_SPLIT[2])
            gpsimd.dma_start(d, sr)._wait_ge(ssem, 16).then_inc(posem, 16)

        @block.vector
        def _(vector):
            # zeros read by the A rects (y in [400, 429)) and the spill
            vector.memset(s[:, 400:432], 0.0).then_inc(ksem, 1)
            # scratch memsets keep DVE busy past the input DMA's sem-value
            # update so the isem wait below polls instead of sleeping
            vector.memset(s[:, 432:616], 0.0)
            vector.memset(s[:, 616:800], 0.0)
            vector.wait_ge(isem, 16)
            vector.tensor_mul(p2[:], th, th)
            vector.drain()
            vector.tensor_mul(p4[:], p2[:], p2[:])
            vector.tensor_scalar(p6[:], p2[:], p2[:, 0:1], p2[:, 0:1],
                                 bass.mybir.AluOpType.mult,
                                 bass.mybir.AluOpType.mult)
            vector.drain()
            vector.tensor_scalar(p16[:], p6[:], p6[:, 0:1], p4[:, 0:1],
                                 bass.mybir.AluOpType.mult,
                                 bass.mybir.AluOpType.mult)
            vector.tensor_scalar_mul(gg[:], c1_c, p6[:, 0:1])
            vector.drain()
            vector.scalar_tensor_tensor(gg[:], c2_c, p16[:, 0:1], gg[:],
                                        bass.mybir.AluOpType.mult,
                                        bass.mybir.AluOpType.add)
            # more scratch memsets: busy-wait DVE up to the exp result so the
            # asem wait polls
            vector.memset(s[:, 800:1000], 0.0)
            vector.memset(s[:, 1000:1193], 0.0)
            vector.wait_ge(asem, 1)
            vector.drain()
            vector.tensor_mul(s[:, KREV:KREV + L], gg[:], ee[:]).then_inc(
                ksem, 1)

    return nc


def _get_nc():
    if "nc" not in _CACHE:
        _CACHE["nc"] = _build_nc()
    return _CACHE["nc"]


def kernel(theta):
    from concourse.bass_utils import run_bass_kernel_spmd

    theta = np.asarray(theta, dtype=np.float32).reshape(B)
    in_maps = [_in_map(theta[c * BPC:(c + 1) * BPC]) for c in range(NCORES)]
    nc = _get_nc()
    res = run_bass_kernel_spmd(nc, in_maps, list(range(NCORES)))
    return np.concatenate([res.results[i]["H"] for i in range(NCORES)], axis=0)
